# revision 7
# baseline (speedup 1.0000x reference)
"""Trainium2 Bass kernel for nn_Encoder_meta (GRU with per-step meta transform).

Reference computation (per batch row b, over T steps):
    x_cat = concat(x_l, x_t, x_w, x_s)            # [B, T, 160]
    x' = tanh(h @ Wt_h.T + x_t @ Wt_x.T + bt)     # [B, 256]
    gx = x' @ W_ih.T + b_ih ; gh = h @ W_hh.T + b_hh
    r = sig(gxr+ghr); z = sig(gxz+ghz); n = tanh(gxn + r*ghn)
    h = (1-z)*n + z*h
Output: h_T as [1, B, H].

Strategy: data-parallel over batch across 8 cores (32 rows each).
Precompute u[t] = x_cat[t] @ Wt_x.T + bt for all t (fp32r matmuls),
store bf16 in SBUF. The serial recurrence runs bf16 matmuls in a
col-tiled batch-stationary layout (partition p = 32*g + b encodes
(hidden-block g, batch row b)).

v3: each gate segment (r, -z, ghn, gxn, xp) accumulates in its OWN
PSUM bank so consumers fire as soon as their segment's matmuls are
done (bank-granular dependency tracking), ordered r -> ghn -> gxn -> z
to overlap the ACT/DVE chain with remaining PE streaming. Biases are
folded in as K=4 selector matmuls that open each bank's accumulation;
u_t is added via an identity-stationary matmul; z weights are negated
so one sigmoid yields w=1-z and h' = h + w*(n-h).
"""

import os
from contextlib import ExitStack

import numpy as np
import ml_dtypes

import concourse.bass as bass
import concourse.mybir as mybir
import concourse.tile as tile
from concourse import bacc
from concourse.bass_utils import run_bass_kernel_spmd

F32 = mybir.dt.float32
F32R = mybir.dt.float32r
BF16 = mybir.dt.bfloat16
BF = ml_dtypes.bfloat16

B, T, H, XP, D = 256, 512, 512, 256, 160
DA = D + 1  # ones row folded in for bt
NCORES, BC = 8, 32  # cores, batch per core
GATE3 = 3 * H  # 1536

Act = mybir.ActivationFunctionType


def _build_nc():
    nc = bacc.Bacc("TRN2", target_bir_lowering=False, debug=False, num_devices=NCORES)

    xT = nc.dram_tensor("xT", [DA, T * BC], F32R, kind="ExternalInput")
    wtx = nc.dram_tensor("wtx", [DA, XP], F32R, kind="ExternalInput")
    wthT = nc.dram_tensor("wthT", [128, 4 * XP], BF16, kind="ExternalInput")
    # whh: per (kt in 4, g in 4) a [128, 384] block [r | -z | n]
    whh = nc.dram_tensor("whh", [128, 16 * 384], BF16, kind="ExternalInput")
    # wih: per (kt in 2, g in 4) a [128, 384] block [n | r | -z]
    wih = nc.dram_tensor("wih", [128, 8 * 384], BF16, kind="ExternalInput")
    # bias rows: per g (row) [b_in | b_r | -b_z | b_hn]
    brow = nc.dram_tensor("brow", [4, 512], BF16, kind="ExternalInput")
    # selector: sel4[k, 32g+b] = (g == k)
    sel4 = nc.dram_tensor("sel4", [4, 128], BF16, kind="ExternalInput")
    hout = nc.dram_tensor("hout", [128, 128], F32, kind="ExternalOutput")

    with tile.TileContext(nc) as tc:
        _kernel_body(tc, xT, wtx, wthT, whh, wih, brow, sel4, hout)
    nc.compile()
    return nc


def _kernel_body(tc, xT, wtx, wthT, whh, wih, brow, sel4, hout):
    nc = tc.nc
    with ExitStack() as ctx:
        const = ctx.enter_context(tc.tile_pool(name="const", bufs=1))

        wthT_sb = const.tile([128, 4 * XP], BF16)
        nc.sync.dma_start(out=wthT_sb[:], in_=wthT.ap())
        whh_sb = const.tile([128, 16 * 384], BF16)
        nc.sync.dma_start(out=whh_sb[:], in_=whh.ap())
        wih_sb = const.tile([128, 8 * 384], BF16)
        nc.sync.dma_start(out=wih_sb[:], in_=wih.ap())
        brow_sb = const.tile([4, 512], BF16)
        nc.sync.dma_start(out=brow_sb[:], in_=brow.ap())
        sel4_sb = const.tile([4, 128], BF16)
        nc.sync.dma_start(out=sel4_sb[:], in_=sel4.ap())
        wtx0_sb = const.tile([128, XP], F32R)
        nc.sync.dma_start(out=wtx0_sb[:], in_=wtx.ap()[0:128, :])
        wtx1_sb = const.tile([DA - 128, XP], F32R)
        nc.sync.dma_start(out=wtx1_sb[:], in_=wtx.ap()[128:DA, :])

        from concourse.masks import make_identity

        ident = const.tile([128, 128], F32)
        make_identity(nc, ident[:])
        ident_bf = const.tile([128, 128], BF16)
        make_identity(nc, ident_bf[:])

        # u[t] in xp-row-transposed layout: [p, t, mt*32 + b], p = xp row % 128
        u_sb = const.tile([128, T, 64], BF16)
        h0_sb = const.tile([128, 128], F32)  # h[32g+b, f] = h[b, 128g+f]
        hT0_sb = const.tile([128, 128], BF16)  # hT[p, 32kt+b] = h[b, 128kt+p]
        hT32_0 = const.tile([128, 128], F32)  # fp32 accumulator for hT
        nc.vector.memset(h0_sb[:], 0.0)
        nc.vector.memset(hT0_sb[:], 0.0)
        nc.vector.memset(hT32_0[:], 0.0)

        # ---- precompute u = [Wt_x | bt] @ [x; 1]  (fp32r, big-N) ----
        with tc.tile_pool(name="xt", bufs=3) as xpool, \
             tc.tile_pool(name="ups", bufs=2, space="PSUM") as ups:
            CW = 512  # (t,b) pairs per fp32r matmul chunk
            NCH = (T * BC) // CW
            SPC = CW // BC  # timesteps per chunk
            for c in range(NCH):
                xt0 = xpool.tile([128, CW], F32R, tag="xt0")
                nc.sync.dma_start(out=xt0[:], in_=xT.ap()[0:128, CW * c : CW * (c + 1)])
                xt1 = xpool.tile([DA - 128, CW], F32R, tag="xt1")
                nc.sync.dma_start(out=xt1[:], in_=xT.ap()[128:DA, CW * c : CW * (c + 1)])
                for mt in range(2):
                    up = ups.tile([128, SPC, 32], F32)
                    nc.tensor.matmul(
                        up[:], wtx0_sb[:, 128 * mt : 128 * (mt + 1)], xt0[:],
                        start=True, stop=False,
                    )
                    nc.tensor.matmul(
                        up[:], wtx1_sb[:, 128 * mt : 128 * (mt + 1)], xt1[:],
                        start=False, stop=True,
                    )
                    nc.vector.tensor_copy(
                        u_sb[:, SPC * c : SPC * (c + 1), 32 * mt : 32 * (mt + 1)], up[:]
                    )

        # ---- recurrence ----
        # per-segment psum banks so readers fire as soon as their segment
        # is accumulated (deps are bank-granular)
        bR = ctx.enter_context(tc.tile_pool(name="bR", bufs=1, space="PSUM"))
        bG = ctx.enter_context(tc.tile_pool(name="bG", bufs=1, space="PSUM"))
        bN = ctx.enter_context(tc.tile_pool(name="bN", bufs=1, space="PSUM"))
        bZ = ctx.enter_context(tc.tile_pool(name="bZ", bufs=1, space="PSUM"))
        bX = ctx.enter_context(tc.tile_pool(name="bX", bufs=2, space="PSUM"))
        tps = ctx.enter_context(tc.tile_pool(name="tps", bufs=1, space="PSUM"))
        work = ctx.enter_context(tc.tile_pool(name="work", bufs=3))

        def bias_mm(ps, col0):
            # ps[32g+b, j] = brow[g, col0+j]; opens the bank's accumulation
            nc.tensor.matmul(
                ps[:, 0:128], sel4_sb[:], brow_sb[:, col0 : col0 + 128],
                start=True, stop=False,
            )

        def gate_mms(ps, lhs, w_sb, nkt, blk_off, stop_last, hT_src):
            # 4g x nkt matmuls of N=128 from whh/wih block column blk_off
            for kt in range(nkt):
                for g in range(4):
                    nc.tensor.matmul(
                        ps[32 * g : 32 * (g + 1), 0:128],
                        (hT_src if lhs is None else lhs)[:, 32 * kt : 32 * (kt + 1)],
                        w_sb[:, (4 * kt + g) * 384 + blk_off :
                             (4 * kt + g) * 384 + blk_off + 128],
                        start=False,
                        stop=(stop_last and kt == nkt - 1),
                        tile_position=(0, 32 * g),
                    )

        h_sb, hT_sb, hT32_sb = h0_sb, hT0_sb, hT32_0
        xp_cur = bX.tile([128, 64], F32, tag="xp")
        nc.tensor.matmul(
            xp_cur[:, 0:64], ident_bf[:], u_sb[:, 0, :], start=True, stop=False
        )
        r_ps = bR.tile([128, 128], F32, tag="r")
        g_ps = bG.tile([128, 128], F32, tag="g")
        n_ps = bN.tile([128, 128], F32, tag="n")
        z_ps = bZ.tile([128, 128], F32, tag="z")
        bias_mm(r_ps, 128)
        bias_mm(g_ps, 384)
        bias_mm(n_ps, 0)
        bias_mm(z_ps, 256)

        for t in range(T):
            # x' = tanh(Wt_h @ h + u_t): weights-stationary, out [xp-row, b]
            for mt in range(2):
                o = xp_cur[:, 32 * mt : 32 * (mt + 1)]
                for kt in range(4):
                    nc.tensor.matmul(
                        o,
                        wthT_sb[:, XP * kt + 128 * mt : XP * kt + 128 * (mt + 1)],
                        hT_sb[:, 32 * kt : 32 * (kt + 1)],
                        start=False,
                        stop=(kt == 3),
                    )
            xp_bf = work.tile([128, 64], BF16, tag="xp_bf")
            nc.scalar.activation(xp_bf[:], xp_cur[:, 0:64], Act.Tanh)

            # r segment first (hT part needs only h; xp part after tanh)
            gate_mms(r_ps, None, whh_sb, 4, 0, False, hT_sb)
            gate_mms(r_ps, xp_bf, wih_sb, 2, 128, True, None)
            r_sb = work.tile([128, 128], F32, tag="r_sb")
            nc.scalar.activation(r_sb[:], r_ps[:], Act.Sigmoid)

            # ghn (recurrent-only), then t1 = r * ghn
            gate_mms(g_ps, None, whh_sb, 4, 256, True, hT_sb)
            t1 = work.tile([128, 128], F32, tag="t1")
            nc.vector.tensor_mul(t1[:], r_sb[:], g_ps[:])

            # gxn (input-only), then t2 = t1 + gxn
            gate_mms(n_ps, xp_bf, wih_sb, 2, 0, True, None)
            t2 = work.tile([128, 128], F32, tag="t2")
            nc.vector.tensor_add(t2[:], t1[:], n_ps[:])

            # -z segment last; overlaps the n-path chain
            gate_mms(z_ps, None, whh_sb, 4, 128, False, hT_sb)
            gate_mms(z_ps, xp_bf, wih_sb, 2, 256, True, None)

            # next step's h-independent psum openers (PE fill during chain)
            if t + 1 < T:
                xp_nxt = bX.tile([128, 64], F32, tag="xp")
                nc.tensor.matmul(
                    xp_nxt[:, 0:64], ident_bf[:], u_sb[:, t + 1, :],
                    start=True, stop=False,
                )
                r_ps2 = bR.tile([128, 128], F32, tag="r")
                g_ps2 = bG.tile([128, 128], F32, tag="g")
                n_ps2 = bN.tile([128, 128], F32, tag="n")
                bias_mm(r_ps2, 128)
                bias_mm(g_ps2, 384)
                bias_mm(n_ps2, 0)

            n_s = work.tile([128, 128], F32, tag="n_s")
            nc.scalar.activation(n_s[:], t2[:], Act.Tanh)
            e_s = work.tile([128, 128], F32, tag="e_s")
            nc.vector.tensor_sub(e_s[:], n_s[:], h_sb[:])
            w_s = work.tile([128, 128], F32, tag="w_s")
            nc.scalar.activation(w_s[:], z_ps[:], Act.Sigmoid)  # w = 1-z
            f_s = work.tile([128, 128], F32, tag="f_s")
            nc.vector.tensor_mul(f_s[:], w_s[:], e_s[:])

            # transpose f (not h): hT' = hT32 + fT lands next step's matmul
            # input in ONE on-path DVE op; h' and the fp32 hT accumulator
            # update run off the critical path.
            fT_ps = tps.tile([128, 128], F32)
            nc.tensor.transpose(fT_ps[:], f_s[:], ident[:])
            hT_new = work.tile([128, 128], BF16, tag="hT")
            nc.vector.tensor_add(hT_new[:], hT32_sb[:], fT_ps[:])
            h_new = work.tile([128, 128], F32, tag="h")
            nc.vector.tensor_add(h_new[:], h_sb[:], f_s[:])
            hT32_new = work.tile([128, 128], F32, tag="hT32")
            nc.vector.tensor_add(hT32_new[:], hT32_sb[:], fT_ps[:])

            h_sb, hT_sb, hT32_sb = h_new, hT_new, hT32_new
            if t + 1 < T:
                z_ps = bZ.tile([128, 128], F32, tag="z")
                bias_mm(z_ps, 256)
                r_ps, g_ps, n_ps = r_ps2, g_ps2, n_ps2
                xp_cur = xp_nxt

        nc.sync.dma_start(out=hout.ap(), in_=h_sb[:])


_CACHE = {}


def _get_nc():
    if "nc" not in _CACHE:
        _CACHE["nc"] = _build_nc()
    return _CACHE["nc"]


def _prep_shared(W_ih, W_hh, b_ih, b_hh, Wt_h, Wt_x, bt):
    wtx = np.ascontiguousarray(
        np.vstack([Wt_x.T, bt[None, :]]).astype(np.float32)
    )  # [161, 256]
    wthT = np.ascontiguousarray(
        Wt_h.T.reshape(4, 128, XP).transpose(1, 0, 2).reshape(128, 4 * XP)
    ).astype(BF)

    WhhT = W_hh.T  # [512, 1536]: cols r | z | n
    blocks = []
    for kt in range(4):
        for g in range(4):
            r_ = WhhT[128 * kt : 128 * (kt + 1), 128 * g : 128 * (g + 1)]
            z_ = -WhhT[128 * kt : 128 * (kt + 1), 512 + 128 * g : 512 + 128 * (g + 1)]
            n_ = WhhT[128 * kt : 128 * (kt + 1), 1024 + 128 * g : 1024 + 128 * (g + 1)]
            blocks.append(np.concatenate([r_, z_, n_], axis=1))
    whh = np.ascontiguousarray(np.concatenate(blocks, axis=1)).astype(BF)

    WihT = W_ih.T  # [256, 1536]
    blocks = []
    for kt in range(2):
        for g in range(4):
            n_ = WihT[128 * kt : 128 * (kt + 1), 1024 + 128 * g : 1024 + 128 * (g + 1)]
            r_ = WihT[128 * kt : 128 * (kt + 1), 128 * g : 128 * (g + 1)]
            z_ = -WihT[128 * kt : 128 * (kt + 1), 512 + 128 * g : 512 + 128 * (g + 1)]
            blocks.append(np.concatenate([n_, r_, z_], axis=1))
    wih = np.ascontiguousarray(np.concatenate(blocks, axis=1)).astype(BF)

    b_rz = b_ih + b_hh
    brow = np.zeros((4, 4, 128), np.float32)
    for g in range(4):
        brow[g, 0, :] = b_ih[1024 + 128 * g : 1024 + 128 * (g + 1)]
        brow[g, 1, :] = b_rz[128 * g : 128 * (g + 1)]
        brow[g, 2, :] = -b_rz[512 + 128 * g : 512 + 128 * (g + 1)]
        brow[g, 3, :] = b_hh[1024 + 128 * g : 1024 + 128 * (g + 1)]
    brow = np.ascontiguousarray(brow.reshape(4, 512)).astype(BF)
    sel4 = np.ascontiguousarray(np.repeat(np.eye(4, dtype=np.float32), 32, axis=1)).astype(BF)
    return wtx, wthT, wih, whh, brow, sel4


def _make_in_maps(x_l_seq, x_t_seq, x_w_seq, x_s_seq, shared):
    wtx, wthT, wih, whh, brow, sel4 = shared
    x_cat = np.concatenate(
        [np.asarray(x_l_seq), np.asarray(x_t_seq), np.asarray(x_w_seq), np.asarray(x_s_seq)],
        axis=-1,
    ).astype(np.float32)  # [B, T, 160]
    in_maps = []
    for c in range(NCORES):
        xc = x_cat[BC * c : BC * (c + 1)]  # [32, T, 160]
        xTc = xc.transpose(2, 1, 0).reshape(D, T * BC)  # [160, t*32+b]
        xTa = np.vstack([xTc, np.ones((1, T * BC), np.float32)])
        in_maps.append(
            {
                "xT": np.ascontiguousarray(xTa),
                "wtx": wtx,
                "wthT": wthT,
                "wih": wih,
                "whh": whh,
                "brow": brow,
                "sel4": sel4,
            }
        )
    return in_maps


def kernel(x_l_seq, x_t_seq, x_w_seq, x_s_seq, W_ih, W_hh, b_ih, b_hh, Wt_h, Wt_x, bt):
    nc = _get_nc()
    shared = _prep_shared(
        np.asarray(W_ih, np.float32), np.asarray(W_hh, np.float32),
        np.asarray(b_ih, np.float32), np.asarray(b_hh, np.float32),
        np.asarray(Wt_h, np.float32), np.asarray(Wt_x, np.float32),
        np.asarray(bt, np.float32),
    )
    in_maps = _make_in_maps(x_l_seq, x_t_seq, x_w_seq, x_s_seq, shared)
    res = run_bass_kernel_spmd(nc, in_maps, core_ids=list(range(NCORES)))
    out = np.zeros((1, B, H), np.float32)
    for c in range(NCORES):
        hc = res.results[c]["hout"]  # [128, 128]
        out[0, BC * c : BC * (c + 1), :] = (
            hc.reshape(4, 32, 128).transpose(1, 0, 2).reshape(32, H)
        )
    return out


# revision 8
# speedup vs baseline: 1.1226x; 1.1226x over previous
"""Trainium2 Bass kernel for nn_Encoder_meta (GRU with per-step meta transform).

Reference computation (per batch row b, over T steps):
    x_cat = concat(x_l, x_t, x_w, x_s)            # [B, T, 160]
    x' = tanh(h @ Wt_h.T + x_t @ Wt_x.T + bt)     # [B, 256]
    gx = x' @ W_ih.T + b_ih ; gh = h @ W_hh.T + b_hh
    r = sig(gxr+ghr); z = sig(gxz+ghz); n = tanh(gxn + r*ghn)
    h = (1-z)*n + z*h
Output: h_T as [1, B, H].

Strategy: data-parallel over batch across 8 cores (32 rows each).
Precompute u[t] = x_cat[t] @ Wt_x.T + bt for all t (fp32r matmuls),
store bf16 in SBUF. The serial recurrence runs bf16 matmuls in a
col-tiled batch-stationary layout (partition p = 32*g + b encodes
(hidden-block g, batch row b)).

v3: each gate segment (r, -z, ghn, gxn, xp) accumulates in its OWN
PSUM bank so consumers fire as soon as their segment's matmuls are
done (bank-granular dependency tracking), ordered r -> ghn -> gxn -> z
to overlap the ACT/DVE chain with remaining PE streaming. Biases are
folded in as K=4 selector matmuls that open each bank's accumulation;
u_t is added via an identity-stationary matmul; z weights are negated
so one sigmoid yields w=1-z and h' = h + w*(n-h).
"""

import os
from contextlib import ExitStack

import numpy as np
import ml_dtypes

import concourse.bass as bass
import concourse.mybir as mybir
import concourse.tile as tile
from concourse import bacc
from concourse.bass_utils import run_bass_kernel_spmd

F32 = mybir.dt.float32
F32R = mybir.dt.float32r
BF16 = mybir.dt.bfloat16
BF = ml_dtypes.bfloat16

B, T, H, XP, D = 256, 512, 512, 256, 160
DA = D + 1  # ones row folded in for bt
NCORES, BC = 8, 32  # cores, batch per core
GATE3 = 3 * H  # 1536

Act = mybir.ActivationFunctionType


def _build_nc():
    nc = bacc.Bacc("TRN2", target_bir_lowering=False, debug=False, num_devices=NCORES)

    xT = nc.dram_tensor("xT", [DA, T * BC], F32R, kind="ExternalInput")
    wtx = nc.dram_tensor("wtx", [DA, XP], F32R, kind="ExternalInput")
    wthT = nc.dram_tensor("wthT", [128, 4 * XP], BF16, kind="ExternalInput")
    # whh: per (kt in 4, g in 4) a [128, 384] block [r | -z | n]
    whh = nc.dram_tensor("whh", [128, 16 * 384], BF16, kind="ExternalInput")
    # wih: per (kt in 2, g in 4) a [128, 384] block [n | r | -z]
    wih = nc.dram_tensor("wih", [128, 8 * 384], BF16, kind="ExternalInput")
    # bias rows: per g (row) [b_in | b_r | -b_z | b_hn]
    brow = nc.dram_tensor("brow", [4, 512], BF16, kind="ExternalInput")
    # selector: sel4[k, 32g+b] = (g == k)
    sel4 = nc.dram_tensor("sel4", [4, 128], BF16, kind="ExternalInput")
    hout = nc.dram_tensor("hout", [128, 128], F32, kind="ExternalOutput")

    with tile.TileContext(nc) as tc:
        _kernel_body(tc, xT, wtx, wthT, whh, wih, brow, sel4, hout)
    nc.compile()
    return nc


def _kernel_body(tc, xT, wtx, wthT, whh, wih, brow, sel4, hout):
    nc = tc.nc
    with ExitStack() as ctx:
        const = ctx.enter_context(tc.tile_pool(name="const", bufs=1))

        wthT_sb = const.tile([128, 4 * XP], BF16)
        nc.sync.dma_start(out=wthT_sb[:], in_=wthT.ap())
        whh_sb = const.tile([128, 16 * 384], BF16)
        nc.sync.dma_start(out=whh_sb[:], in_=whh.ap())
        wih_sb = const.tile([128, 8 * 384], BF16)
        nc.sync.dma_start(out=wih_sb[:], in_=wih.ap())
        brow_sb = const.tile([4, 512], BF16)
        nc.sync.dma_start(out=brow_sb[:], in_=brow.ap())
        sel4_sb = const.tile([4, 128], BF16)
        nc.sync.dma_start(out=sel4_sb[:], in_=sel4.ap())
        wtx0_sb = const.tile([128, XP], F32R)
        nc.sync.dma_start(out=wtx0_sb[:], in_=wtx.ap()[0:128, :])
        wtx1_sb = const.tile([DA - 128, XP], F32R)
        nc.sync.dma_start(out=wtx1_sb[:], in_=wtx.ap()[128:DA, :])

        from concourse.masks import make_identity

        ident = const.tile([128, 128], F32)
        make_identity(nc, ident[:])
        ident_bf = const.tile([128, 128], BF16)
        make_identity(nc, ident_bf[:])

        # u[t] in xp-row-transposed layout: [p, t, mt*32 + b], p = xp row % 128
        u_sb = const.tile([128, T, 64], BF16)
        h0_sb = const.tile([128, 128], F32)  # h[32g+b, f] = h[b, 128g+f]
        hT0_sb = const.tile([128, 128], BF16)  # hT[p, 32kt+b] = h[b, 128kt+p]
        nc.vector.memset(h0_sb[:], 0.0)
        nc.vector.memset(hT0_sb[:], 0.0)

        # ---- precompute u = [Wt_x | bt] @ [x; 1]  (fp32r, big-N) ----
        with tc.tile_pool(name="xt", bufs=3) as xpool, \
             tc.tile_pool(name="ups", bufs=2, space="PSUM") as ups:
            CW = 512  # (t,b) pairs per fp32r matmul chunk
            NCH = (T * BC) // CW
            SPC = CW // BC  # timesteps per chunk
            for c in range(NCH):
                xt0 = xpool.tile([128, CW], F32R, tag="xt0")
                nc.sync.dma_start(out=xt0[:], in_=xT.ap()[0:128, CW * c : CW * (c + 1)])
                xt1 = xpool.tile([DA - 128, CW], F32R, tag="xt1")
                nc.sync.dma_start(out=xt1[:], in_=xT.ap()[128:DA, CW * c : CW * (c + 1)])
                for mt in range(2):
                    up = ups.tile([128, SPC, 32], F32)
                    nc.tensor.matmul(
                        up[:], wtx0_sb[:, 128 * mt : 128 * (mt + 1)], xt0[:],
                        start=True, stop=False,
                    )
                    nc.tensor.matmul(
                        up[:], wtx1_sb[:, 128 * mt : 128 * (mt + 1)], xt1[:],
                        start=False, stop=True,
                    )
                    nc.vector.tensor_copy(
                        u_sb[:, SPC * c : SPC * (c + 1), 32 * mt : 32 * (mt + 1)], up[:]
                    )

        # ---- recurrence ----
        # per-segment psum banks so readers fire as soon as their segment
        # is accumulated (deps are bank-granular)
        bR = ctx.enter_context(tc.tile_pool(name="bR", bufs=1, space="PSUM"))
        bG = ctx.enter_context(tc.tile_pool(name="bG", bufs=1, space="PSUM"))
        bN = ctx.enter_context(tc.tile_pool(name="bN", bufs=1, space="PSUM"))
        bZ = ctx.enter_context(tc.tile_pool(name="bZ", bufs=1, space="PSUM"))
        bX = ctx.enter_context(tc.tile_pool(name="bX", bufs=2, space="PSUM"))
        tps = ctx.enter_context(tc.tile_pool(name="tps", bufs=1, space="PSUM"))
        work = ctx.enter_context(tc.tile_pool(name="work", bufs=3))

        def bias_mm(ps, col0):
            # ps[32g+b, j] = brow[g, col0+j]; opens the bank's accumulation
            nc.tensor.matmul(
                ps[:, 0:128], sel4_sb[:], brow_sb[:, col0 : col0 + 128],
                start=True, stop=False,
            )

        def gate_mms(ps, lhs, w_sb, nkt, blk_off, stop_last, hT_src):
            # 4g x nkt matmuls of N=128 from whh/wih block column blk_off
            for kt in range(nkt):
                for g in range(4):
                    nc.tensor.matmul(
                        ps[32 * g : 32 * (g + 1), 0:128],
                        (hT_src if lhs is None else lhs)[:, 32 * kt : 32 * (kt + 1)],
                        w_sb[:, (4 * kt + g) * 384 + blk_off :
                             (4 * kt + g) * 384 + blk_off + 128],
                        start=False,
                        stop=(stop_last and kt == nkt - 1),
                        tile_position=(0, 32 * g),
                    )

        h_sb, hT_sb = h0_sb, hT0_sb
        xp_cur = bX.tile([128, 64], F32, tag="xp")
        nc.tensor.matmul(
            xp_cur[:, 0:64], ident_bf[:], u_sb[:, 0, :], start=True, stop=False
        )
        r_ps = bR.tile([128, 128], F32, tag="r")
        g_ps = bG.tile([128, 128], F32, tag="g")
        n_ps = bN.tile([128, 128], F32, tag="n")
        z_ps = bZ.tile([128, 128], F32, tag="z")
        bias_mm(r_ps, 128)
        bias_mm(g_ps, 384)
        bias_mm(n_ps, 0)
        bias_mm(z_ps, 256)

        for t in range(T):
            # x' = tanh(Wt_h @ h + u_t): weights-stationary, out [xp-row, b]
            for mt in range(2):
                o = xp_cur[:, 32 * mt : 32 * (mt + 1)]
                for kt in range(4):
                    nc.tensor.matmul(
                        o,
                        wthT_sb[:, XP * kt + 128 * mt : XP * kt + 128 * (mt + 1)],
                        hT_sb[:, 32 * kt : 32 * (kt + 1)],
                        start=False,
                        stop=(kt == 3),
                    )
            xp_bf = work.tile([128, 64], BF16, tag="xp_bf")
            nc.scalar.activation(xp_bf[:], xp_cur[:, 0:64], Act.Tanh)

            # r segment first (hT part needs only h; xp part after tanh)
            gate_mms(r_ps, None, whh_sb, 4, 0, False, hT_sb)
            gate_mms(r_ps, xp_bf, wih_sb, 2, 128, True, None)
            r_sb = work.tile([128, 128], F32, tag="r_sb")
            nc.scalar.activation(r_sb[:], r_ps[:], Act.Sigmoid)

            # ghn (recurrent-only), then t1 = r * ghn
            gate_mms(g_ps, None, whh_sb, 4, 256, True, hT_sb)
            t1 = work.tile([128, 128], F32, tag="t1")
            nc.vector.tensor_mul(t1[:], r_sb[:], g_ps[:])

            # gxn (input-only), then t2 = t1 + gxn
            gate_mms(n_ps, xp_bf, wih_sb, 2, 0, True, None)
            t2 = work.tile([128, 128], F32, tag="t2")
            nc.vector.tensor_add(t2[:], t1[:], n_ps[:])

            # -z segment last; overlaps the n-path chain
            gate_mms(z_ps, None, whh_sb, 4, 128, False, hT_sb)
            gate_mms(z_ps, xp_bf, wih_sb, 2, 256, True, None)

            # next step's h-independent psum openers (PE fill during chain)
            if t + 1 < T:
                xp_nxt = bX.tile([128, 64], F32, tag="xp")
                nc.tensor.matmul(
                    xp_nxt[:, 0:64], ident_bf[:], u_sb[:, t + 1, :],
                    start=True, stop=False,
                )
                r_ps2 = bR.tile([128, 128], F32, tag="r")
                g_ps2 = bG.tile([128, 128], F32, tag="g")
                n_ps2 = bN.tile([128, 128], F32, tag="n")
                bias_mm(r_ps2, 128)
                bias_mm(g_ps2, 384)
                bias_mm(n_ps2, 0)

            n_s = work.tile([128, 128], F32, tag="n_s")
            nc.scalar.activation(n_s[:], t2[:], Act.Tanh)
            e_s = work.tile([128, 128], F32, tag="e_s")
            nc.vector.tensor_sub(e_s[:], n_s[:], h_sb[:])
            w_s = work.tile([128, 128], F32, tag="w_s")
            nc.scalar.activation(w_s[:], z_ps[:], Act.Sigmoid)  # w = 1-z
            f_s = work.tile([128, 128], F32, tag="f_s")
            nc.vector.tensor_mul(f_s[:], w_s[:], e_s[:])
            h_new = work.tile([128, 128], F32, tag="h")
            nc.vector.tensor_add(h_new[:], h_sb[:], f_s[:])

            # hT for next step: one full 128x128 PE transpose
            hT_ps = tps.tile([128, 128], F32)
            nc.tensor.transpose(hT_ps[:], h_new[:], ident[:])
            hT_new = work.tile([128, 128], BF16, tag="hT")
            nc.vector.tensor_copy(hT_new[:], hT_ps[:])

            h_sb, hT_sb = h_new, hT_new
            if t + 1 < T:
                z_ps = bZ.tile([128, 128], F32, tag="z")
                bias_mm(z_ps, 256)
                r_ps, g_ps, n_ps = r_ps2, g_ps2, n_ps2
                xp_cur = xp_nxt

        nc.sync.dma_start(out=hout.ap(), in_=h_sb[:])


_CACHE = {}


def _get_nc():
    if "nc" not in _CACHE:
        _CACHE["nc"] = _build_nc()
    return _CACHE["nc"]


def _prep_shared(W_ih, W_hh, b_ih, b_hh, Wt_h, Wt_x, bt):
    wtx = np.ascontiguousarray(
        np.vstack([Wt_x.T, bt[None, :]]).astype(np.float32)
    )  # [161, 256]
    wthT = np.ascontiguousarray(
        Wt_h.T.reshape(4, 128, XP).transpose(1, 0, 2).reshape(128, 4 * XP)
    ).astype(BF)

    WhhT = W_hh.T  # [512, 1536]: cols r | z | n
    blocks = []
    for kt in range(4):
        for g in range(4):
            r_ = WhhT[128 * kt : 128 * (kt + 1), 128 * g : 128 * (g + 1)]
            z_ = -WhhT[128 * kt : 128 * (kt + 1), 512 + 128 * g : 512 + 128 * (g + 1)]
            n_ = WhhT[128 * kt : 128 * (kt + 1), 1024 + 128 * g : 1024 + 128 * (g + 1)]
            blocks.append(np.concatenate([r_, z_, n_], axis=1))
    whh = np.ascontiguousarray(np.concatenate(blocks, axis=1)).astype(BF)

    WihT = W_ih.T  # [256, 1536]
    blocks = []
    for kt in range(2):
        for g in range(4):
            n_ = WihT[128 * kt : 128 * (kt + 1), 1024 + 128 * g : 1024 + 128 * (g + 1)]
            r_ = WihT[128 * kt : 128 * (kt + 1), 128 * g : 128 * (g + 1)]
            z_ = -WihT[128 * kt : 128 * (kt + 1), 512 + 128 * g : 512 + 128 * (g + 1)]
            blocks.append(np.concatenate([n_, r_, z_], axis=1))
    wih = np.ascontiguousarray(np.concatenate(blocks, axis=1)).astype(BF)

    b_rz = b_ih + b_hh
    brow = np.zeros((4, 4, 128), np.float32)
    for g in range(4):
        brow[g, 0, :] = b_ih[1024 + 128 * g : 1024 + 128 * (g + 1)]
        brow[g, 1, :] = b_rz[128 * g : 128 * (g + 1)]
        brow[g, 2, :] = -b_rz[512 + 128 * g : 512 + 128 * (g + 1)]
        brow[g, 3, :] = b_hh[1024 + 128 * g : 1024 + 128 * (g + 1)]
    brow = np.ascontiguousarray(brow.reshape(4, 512)).astype(BF)
    sel4 = np.ascontiguousarray(np.repeat(np.eye(4, dtype=np.float32), 32, axis=1)).astype(BF)
    return wtx, wthT, wih, whh, brow, sel4


def _make_in_maps(x_l_seq, x_t_seq, x_w_seq, x_s_seq, shared):
    wtx, wthT, wih, whh, brow, sel4 = shared
    x_cat = np.concatenate(
        [np.asarray(x_l_seq), np.asarray(x_t_seq), np.asarray(x_w_seq), np.asarray(x_s_seq)],
        axis=-1,
    ).astype(np.float32)  # [B, T, 160]
    in_maps = []
    for c in range(NCORES):
        xc = x_cat[BC * c : BC * (c + 1)]  # [32, T, 160]
        xTc = xc.transpose(2, 1, 0).reshape(D, T * BC)  # [160, t*32+b]
        xTa = np.vstack([xTc, np.ones((1, T * BC), np.float32)])
        in_maps.append(
            {
                "xT": np.ascontiguousarray(xTa),
                "wtx": wtx,
                "wthT": wthT,
                "wih": wih,
                "whh": whh,
                "brow": brow,
                "sel4": sel4,
            }
        )
    return in_maps


def kernel(x_l_seq, x_t_seq, x_w_seq, x_s_seq, W_ih, W_hh, b_ih, b_hh, Wt_h, Wt_x, bt):
    nc = _get_nc()
    shared = _prep_shared(
        np.asarray(W_ih, np.float32), np.asarray(W_hh, np.float32),
        np.asarray(b_ih, np.float32), np.asarray(b_hh, np.float32),
        np.asarray(Wt_h, np.float32), np.asarray(Wt_x, np.float32),
        np.asarray(bt, np.float32),
    )
    in_maps = _make_in_maps(x_l_seq, x_t_seq, x_w_seq, x_s_seq, shared)
    res = run_bass_kernel_spmd(nc, in_maps, core_ids=list(range(NCORES)))
    out = np.zeros((1, B, H), np.float32)
    for c in range(NCORES):
        hc = res.results[c]["hout"]  # [128, 128]
        out[0, BC * c : BC * (c + 1), :] = (
            hc.reshape(4, 32, 128).transpose(1, 0, 2).reshape(32, H)
        )
    return out
